# revision 2
# baseline (speedup 1.0000x reference)
"""MultiHeadAttention (1x1-conv projections) Trainium2 Bass kernel.

Problem: x[8,256,32,32]; q/k/v = conv1x1(x, W*, b*); 8 heads, dk=dv=32;
attention over N=H*W=1024 positions; out = conv1x1(o, Wo, bo).

Sharding: data-parallel over batch -- core c computes batch c.

Per-core dataflow (everything stays on-chip after the initial loads):
  X [256,1024] (C on partitions, 2 tiles of 128)
  q = Wq@X+bq, k = Wk@X+bk       -> [co_part, n]   (co = 32*head+d)
  vT = (Wv@X)^T via x-stationary -> [n_part, co] in BF16 with a ones
       column per head, so the PV matmul also produces the softmax
       denominator for free
  per head pair: S^T[nk,nq] = k_h^T q_h (K=dk=32; the two heads run
       concurrently in distinct PE row strips via tile_position);
       P^T = exp(scale*S^T) -> BF16, split across TWO engines:
         - ScalarE: table exp (ACT), ~1 elem/cycle/lane @1.2GHz
         - VectorE: bf16 Schraudolph in ONE tensor_scalar op:
             i16 = int(z*(2^7*log2e*scale) + (127*2^7 - 5.5));
           the int16 bit pattern read as bf16 IS 2^(z*log2e) with
           linearly-interpolated mantissa (max rel err ~3.3%, fine for
           the 2e-2 gate; softmax normalization cancels most of it)
       o_h[dv+1, nq] = [vT_h|1]^T P^T accumulated over nk tiles in PSUM;
       the two heads' PV matmuls run CONCURRENTLY via column tiling
       (tile_position (0,0) / (0,64), disjoint col-groups)
  o_norm = o / denom ; y = Wo@o_norm + (Wo@bv + bo)

Softmax max-subtraction is skipped: logits ~ N(0,1) so exp() cannot
overflow, and softmax is shift-invariant. bv is folded into the output
bias (Wo@bv + bo), computed on-device.

Normalize epilogue per pass: 2 DVE reciprocals (PSUM-direct) -> rec2
[2,512]; one K=2 selector matmul broadcasts both heads' reciprocal rows
across their 32-partition strips; 2 ScalarE copies evacuate raw o; one
DVE tensor_tensor does the [64,512] multiply.

QK/projections run as float32r (full-rate fp32); PV runs BF16.

PSUM budget (8 banks): qk 2x[128,1024]=4 (double-buffered so QK overlaps
exp on both engines), pv 3x[128,512]=3 (accumulate + prev-pass epilogue +
rotation), shared [128,512] slot (projections / rec broadcast / output
projection) = 1.

All engines execute their streams strictly in-order, so the emission is
software-pipelined by hand: each pass's last two PV pairs and its
normalize epilogue are emitted inside the NEXT pass, q/k/v projections
are interleaved into the seams, and each nq-half's output projection +
store overlap the other half's attention.
"""

import numpy as np

import concourse.bass as bass
import concourse.bacc as bacc
import concourse.mybir as mybir
import concourse.tile as tile
from concourse.bass_utils import run_bass_kernel_spmd

F32 = mybir.dt.float32
F32R = mybir.dt.float32r
BF16 = mybir.dt.bfloat16
I16 = mybir.dt.int16
AF = mybir.ActivationFunctionType

P = 128
C = 256          # channels (= Ck = Cv = Co)
CT = 2           # channel tiles of 128
N = 1024         # sequence length (H*W)
NH = 8           # heads
DK = 32          # head dim
SCALE = DK ** -0.5
NQH = 2          # nq halves (512 each; fp32 matmul free-dim limit)
NKT = 8          # nk tiles of 128

LOG2E = 1.4426950408889634
A16 = float(2.0 ** 7 * LOG2E * SCALE)   # schraudolph slope, softmax scale folded
B16 = float(127.0 * 2 ** 7 - 5.5)       # schraudolph offset, C=5.5 tuned

# which nk tiles of each pass run the exp on VectorE (bf16 Schraudolph);
# the rest run on ScalarE (table exp). 26/64 on DVE balances the engines.
DVE_NKS = (
    (1, 3, 5), (1, 3, 5, 7), (1, 3, 5), (1, 3, 5, 7),
    (1, 3, 5), (1, 3, 5, 7), (1, 3, 5), (1, 3, 5, 7),
)


def build_nc(reps=1, pipelined=True):
    nc = bacc.Bacc(None, target_bir_lowering=False, debug=False)

    x_d = nc.dram_tensor("x", [C, N], F32R, kind="ExternalInput")
    wqt_d = nc.dram_tensor("wqt", [C, C], F32R, kind="ExternalInput")
    wkt_d = nc.dram_tensor("wkt", [C, C], F32R, kind="ExternalInput")
    wvt_d = nc.dram_tensor("wvt", [C, C], F32R, kind="ExternalInput")
    wot_d = nc.dram_tensor("wot", [C, C], F32R, kind="ExternalInput")
    bq_d = nc.dram_tensor("bq", [C], F32, kind="ExternalInput")
    bk_d = nc.dram_tensor("bk", [C], F32, kind="ExternalInput")
    bv_d = nc.dram_tensor("bv", [C], F32R, kind="ExternalInput")
    bo_d = nc.dram_tensor("bo", [C], F32, kind="ExternalInput")
    y_d = nc.dram_tensor("y", [C, N], F32, kind="ExternalOutput")

    with tile.TileContext(nc) as tc:
        with (
            tc.tile_pool(name="const", bufs=1) as cpool,
            tc.tile_pool(name="work", bufs=1) as wpool,
            tc.tile_pool(name="qkpsum", bufs=2, space="PSUM") as qkpool,
            tc.tile_pool(name="pvpsum", bufs=3, space="PSUM") as pvpool,
            tc.tile_pool(name="mmpsum", bufs=1, space="PSUM") as mmpool,
            tc.tile_pool(name="ptpool", bufs=6) as ptpool,
            tc.tile_pool(name="eppool", bufs=3) as eppool,
            tc.tile_pool(name="recpool", bufs=3) as recpool,
        ):
            # ---- loads ----
            w_s = {}
            for name, d in (("q", wqt_d), ("k", wkt_d), ("v", wvt_d), ("o", wot_d)):
                w_s[name] = cpool.tile([P, CT, C], F32R, tag=f"w{name}",
                                       name=f"w{name}")
            x_s = cpool.tile([P, CT, N], F32R)
            xr = x_d[:].rearrange("(t p) n -> p t n", p=P)
            for nh in range(NQH):
                nc.sync.dma_start(
                    x_s[:, 0, nh * 512 : (nh + 1) * 512],
                    xr[:, 0, nh * 512 : (nh + 1) * 512],
                )
                nc.scalar.dma_start(
                    x_s[:, 1, nh * 512 : (nh + 1) * 512],
                    xr[:, 1, nh * 512 : (nh + 1) * 512],
                )
            nc.gpsimd.dma_start(
                w_s["q"][:], wqt_d[:].rearrange("(t p) c -> p t c", p=P)
            )
            nc.gpsimd.dma_start(
                w_s["k"][:], wkt_d[:].rearrange("(t p) c -> p t c", p=P)
            )
            b_s = {}
            for name, d in (("q", bq_d), ("k", bk_d), ("o", bo_d)):
                b_s[name] = cpool.tile([P, CT], F32, tag=f"b{name}",
                                       name=f"b{name}")
                nc.gpsimd.dma_start(b_s[name][:], d[:].rearrange("(t p) -> p t", p=P))
            nc.gpsimd.dma_start(
                w_s["v"][:], wvt_d[:].rearrange("(t p) c -> p t c", p=P)
            )
            nc.gpsimd.dma_start(
                w_s["o"][:], wot_d[:].rearrange("(t p) c -> p t c", p=P)
            )
            # bv duplicated along a free dim of 2: fp32r matmuls need N>=2
            bv2_s = cpool.tile([P, CT, 2], F32R, tag="bv2")
            for j in range(2):
                nc.gpsimd.dma_start(
                    bv2_s[:, :, j], bv_d[:].rearrange("(t p) -> p t", p=P)
                )

            # ---- persistent working tiles ----
            q_s = wpool.tile([P, CT, N], F32R, tag="q")     # [co_p, co_t, n]
            k_s = wpool.tile([P, CT, N], F32R, tag="k")
            # vT in BF16 with a ones column per head: [n_p, n_t, head, dv+1]
            vt_s = wpool.tile([P, NKT, NH, DK + 1], BF16, tag="vt")
            o_s = wpool.tile([P, CT, N], F32R, tag="o")     # normalized attn out
            y_s = wpool.tile([P, CT, N], F32, tag="y")
            bo2_s = wpool.tile([P, CT], F32, tag="bo2")     # Wo@bv + bo

            nc.vector.memset(vt_s[:, :, :, DK : DK + 1], 1.0)
            # selector for the reciprocal broadcast: bcp[0:64] = sel2^T @ rec2
            sel2 = wpool.tile([2, 64], F32R, tag="sel2")
            nc.vector.memset(sel2[:].bitcast(F32), 0.0)
            nc.vector.memset(sel2[0:1, 0:DK].bitcast(F32), 1.0)
            nc.vector.memset(sel2[1:2, DK : 2 * DK].bitcast(F32), 1.0)
            ones1 = wpool.tile([1, 2], F32, tag="ones1")
            nc.vector.memset(ones1[:], 1.0)
            # pull the exp ACT-table load into the load phase
            warm = wpool.tile([1, 2], F32, tag="warm")
            nc.scalar.activation(warm[:], ones1[:], AF.Exp)

            # ---- emission helpers ----
            def qk_proj(name, dst, ct, nh):
                ps = mmpool.tile([P, 512], F32, tag="mm512", name="ps")
                for ci in range(CT):
                    nc.tensor.matmul(
                        ps[:],
                        w_s[name][:, ci, ct * P : (ct + 1) * P],
                        x_s[:, ci, nh * 512 : (nh + 1) * 512],
                        start=(ci == 0),
                        stop=(ci == CT - 1),
                    )
                nc.scalar.activation(
                    dst[:, ct, nh * 512 : (nh + 1) * 512],
                    ps[:],
                    AF.Identity,
                    bias=b_s[name][:, ct : ct + 1],
                )

            def v_proj(nt):
                ps = mmpool.tile([P, 512], F32, tag="mm512", name="ps")
                for ci in range(CT):
                    nc.tensor.matmul(
                        ps[:, 0:C],
                        x_s[:, ci, nt * P : (nt + 1) * P],
                        w_s["v"][:, ci, :],
                        start=(ci == 0),
                        stop=(ci == CT - 1),
                    )
                nc.vector.tensor_copy(
                    vt_s[:, nt, :, 0:DK],
                    ps[:, 0:C].rearrange("p (h d) -> p h d", d=DK),
                )

            def bo2_proj():
                # bo2 = WoT.T @ bv + bo
                for ct in range(CT):
                    ps = mmpool.tile([P, 512], F32, tag="mm512", name="ps")
                    for ci in range(CT):
                        nc.tensor.matmul(
                            ps[:, 0:2],
                            w_s["o"][:, ci, ct * P : (ct + 1) * P],
                            bv2_s[:, ci, :],
                            start=(ci == 0),
                            stop=(ci == CT - 1),
                        )
                    nc.vector.tensor_scalar_add(
                        bo2_s[:, ct : ct + 1], ps[:, 0:1],
                        b_s["o"][:, ct : ct + 1]
                    )

            def pv_pair(pvs, nk, hg, hl0, pt, start, stop):
                # the two heads' PV matmuls occupy disjoint col-groups
                # ({0,1} vs {2,3}) so they run concurrently on the PE
                nc.tensor.matmul(
                    pvs[0 : DK + 1, :],
                    vt_s[:, nk, 4 * hg + hl0, :],
                    pt[:, 0:512],
                    start=start,
                    stop=stop,
                )
                nc.tensor.matmul(
                    pvs[64 : 64 + DK + 1, :],
                    vt_s[:, nk, 4 * hg + hl0 + 1, :],
                    pt[:, 512:1024],
                    start=start,
                    stop=stop,
                    tile_position=(0, 64),
                )

            def attn_pass(nqh, pp, pass_idx, pre_pv=None, tail_prev=None,
                          epi_prev=None, last=False):
                """One pass = 2 heads (4*hg + hl0, +1) x one nq-half.

                The engine streams are strictly in-order, so anything gated
                on this pass's LAST exps is emitted inside the NEXT pass:
                the last two nk's PV pairs as `pv_tail` (at nk==0), the
                normalize epilogue as `epilogue` (at nk==3).
                """
                hg = pp // 2
                hl0 = (pp % 2) * 2
                dve_nks = DVE_NKS[pass_idx]
                pvs = pvpool.tile([P, 512], F32, tag="pv", name="pvs")
                pt_hold = {}
                for nk in range(NKT):
                    qk = qkpool.tile([P, 1024], F32, tag="qk")
                    for j in range(2):
                        hl = hl0 + j
                        nc.tensor.matmul(
                            qk[:, j * 512 : (j + 1) * 512],
                            k_s[hl * DK : (hl + 1) * DK, hg,
                                nk * P : (nk + 1) * P],
                            q_s[hl * DK : (hl + 1) * DK, hg,
                                nqh * 512 : (nqh + 1) * 512],
                            start=True,
                            stop=True,
                            tile_position=(hl * DK, 0),
                        )
                    pt = ptpool.tile([P, 1024], BF16, tag="pt")
                    if nk in dve_nks:
                        with nc.allow_low_precision(reason="bf16 schraudolph"):
                            nc.vector.tensor_scalar(
                                pt[:].bitcast(I16), qk[:], A16, B16,
                                mybir.AluOpType.mult, mybir.AluOpType.add,
                            )
                    else:
                        nc.scalar.activation(pt[:], qk[:], AF.Exp, scale=SCALE)
                    if nk == 0 and tail_prev is not None:
                        tail_prev()
                    if nk == 3 and epi_prev is not None:
                        epi_prev()
                    if pre_pv is not None:
                        pre_pv(nk)
                    if nk < NKT - 2:
                        pv_pair(pvs, nk, hg, hl0, pt, start=(nk == 0),
                                stop=False)
                    else:
                        pt_hold[nk] = pt

                def pv_tail():
                    for nk in (NKT - 2, NKT - 1):
                        pv_pair(pvs, nk, hg, hl0, pt_hold[nk], start=False,
                                stop=(nk == NKT - 1))

                def epilogue():
                    # denominators live in PSUM rows 32 / 96; reciprocal them
                    # into adjacent partitions, broadcast via a K=2 selector
                    # matmul, evacuate raw o on ScalarE, multiply on DVE.
                    rec2 = recpool.tile([2, 512], F32R, tag="rec", name="rec2")
                    with nc.allow_low_precision(reason="f32r recip"):
                        nc.vector.reciprocal(rec2[0:1, :], pvs[DK : DK + 1, :])
                        nc.vector.reciprocal(
                            rec2[1:2, :], pvs[64 + DK : 64 + DK + 1, :]
                        )
                    oraw = eppool.tile([64, 512], F32, tag="oraw", name="oraw")
                    nc.scalar.copy(oraw[0:DK, :], pvs[0:DK, :])
                    nc.scalar.copy(oraw[DK : 2 * DK, :], pvs[64 : 64 + DK, :])
                    bcp = mmpool.tile([P, 512], F32, tag="mm512", name="bcp")
                    nc.tensor.matmul(
                        bcp[0:64, :], sel2[:], rec2[:], start=True, stop=True
                    )
                    nc.vector.tensor_tensor(
                        o_s[hl0 * DK : hl0 * DK + 2 * DK, hg,
                            nqh * 512 : (nqh + 1) * 512],
                        oraw[:],
                        bcp[0:64, :],
                        mybir.AluOpType.mult,
                    )

                return pv_tail, epilogue

            def out_proj(nqh, last=False):
                for ct in range(CT):
                    ps = mmpool.tile([P, 512], F32, tag="mm512", name="ps")
                    for i, cv in enumerate((1, 0)):
                        nc.tensor.matmul(
                            ps[:],
                            w_s["o"][:, cv, ct * P : (ct + 1) * P],
                            o_s[:, cv, nqh * 512 : (nqh + 1) * 512],
                            start=(i == 0),
                            stop=(i == CT - 1),
                        )
                    if last:
                        nc.scalar.activation(
                            y_s[:, ct, nqh * 512 : (nqh + 1) * 512],
                            ps[:],
                            AF.Identity,
                            bias=bo2_s[:, ct : ct + 1],
                        )
                    else:
                        nc.vector.tensor_scalar_add(
                            y_s[:, ct, nqh * 512 : (nqh + 1) * 512],
                            ps[:],
                            bo2_s[:, ct : ct + 1],
                        )
                    nc.sync.dma_start(
                        y_d[:].rearrange("(t p) n -> p t n", p=P)[
                            :, ct, nqh * 512 : (nqh + 1) * 512
                        ],
                        y_s[:, ct, nqh * 512 : (nqh + 1) * 512],
                    )

            # ---- emission order: overlap projections with attention ----
            for _rep in range(reps):
                qk_proj("q", q_s, 1, 0)
                qk_proj("k", k_s, 1, 0)
                qk_proj("k", k_s, 1, 1)
                tailp, epip = attn_pass(0, 2, 0, pre_pv=v_proj)
                qk_proj("q", q_s, 0, 0)
                qk_proj("k", k_s, 0, 0)
                tailp, epip = attn_pass(0, 3, 1, tail_prev=tailp, epi_prev=epip)
                qk_proj("k", k_s, 0, 1)
                qk_proj("q", q_s, 1, 1)
                tailp, epip = attn_pass(0, 0, 2, tail_prev=tailp, epi_prev=epip)
                qk_proj("q", q_s, 0, 1)
                bo2_proj()
                tailp, epip = attn_pass(0, 1, 3, tail_prev=tailp, epi_prev=epip)

                def epi_and_oproj0(epip=epip):
                    epip()
                    out_proj(0)

                tailp, epip = attn_pass(1, 2, 4, tail_prev=tailp,
                                        epi_prev=epi_and_oproj0)
                tailp, epip = attn_pass(1, 3, 5, tail_prev=tailp, epi_prev=epip)
                tailp, epip = attn_pass(1, 0, 6, tail_prev=tailp, epi_prev=epip)
                tailp, epip = attn_pass(1, 1, 7, tail_prev=tailp,
                                        epi_prev=epip, last=True)
                tailp()
                epip()
                out_proj(1, last=True)
    nc.compile()
    return nc


_NC = None


def _get_nc():
    global _NC
    if _NC is None:
        _NC = build_nc()
    return _NC


def make_in_maps(x, Wq, bq, Wk, bk, Wv, bv, Wo, bo):
    B = x.shape[0]
    xs = np.ascontiguousarray(x.reshape(B, C, N).astype(np.float32, copy=False))
    shared = {
        "wqt": np.ascontiguousarray(Wq.T.astype(np.float32, copy=False)),
        "wkt": np.ascontiguousarray(Wk.T.astype(np.float32, copy=False)),
        "wvt": np.ascontiguousarray(Wv.T.astype(np.float32, copy=False)),
        "wot": np.ascontiguousarray(Wo.T.astype(np.float32, copy=False)),
        "bq": np.ascontiguousarray(bq.astype(np.float32, copy=False)),
        "bk": np.ascontiguousarray(bk.astype(np.float32, copy=False)),
        "bv": np.ascontiguousarray(bv.astype(np.float32, copy=False)),
        "bo": np.ascontiguousarray(bo.astype(np.float32, copy=False)),
    }
    return [dict(shared, x=xs[c]) for c in range(B)]


def kernel(x, Wq, bq, Wk, bk, Wv, bv, Wo, bo, **run_kwargs):
    x = np.asarray(x)
    B, _, H, W = x.shape
    in_maps = make_in_maps(
        x, np.asarray(Wq), np.asarray(bq), np.asarray(Wk), np.asarray(bk),
        np.asarray(Wv), np.asarray(bv), np.asarray(Wo), np.asarray(bo),
    )
    res = run_bass_kernel_spmd(_get_nc(), in_maps, core_ids=list(range(B)),
                               **run_kwargs)
    y = np.stack([res.results[c]["y"] for c in range(B)])
    out = y.reshape(B, C, H, W)
    if run_kwargs:
        kernel.last_result = res
    return out


# revision 16
# speedup vs baseline: 1.0020x; 1.0020x over previous
"""MultiHeadAttention (1x1-conv projections) Trainium2 Bass kernel.

Problem: x[8,256,32,32]; q/k/v = conv1x1(x, W*, b*); 8 heads, dk=dv=32;
attention over N=H*W=1024 positions; out = conv1x1(o, Wo, bo).

Sharding: data-parallel over batch -- core c computes batch c.

Per-core dataflow (everything stays on-chip after the initial loads):
  X [256,1024] (C on partitions, 2 tiles of 128)
  q = Wq@X+bq, k = Wk@X+bk       -> [co_part, n]   (co = 32*head+d)
  vT = (Wv@X)^T via x-stationary -> [n_part, co] in BF16 with a ones
       column per head, so the PV matmul also produces the softmax
       denominator for free
  per head pair: S^T[nk,nq] = k_h^T q_h (K=dk=32; the two heads run
       concurrently in distinct PE row strips via tile_position);
       P^T = exp(scale*S^T) -> BF16, split across TWO engines:
         - ScalarE: table exp (ACT), ~1 elem/cycle/lane @1.2GHz
         - VectorE: bf16 Schraudolph in ONE tensor_scalar op:
             i16 = int(z*(2^7*log2e*scale) + (127*2^7 - 5.5));
           the int16 bit pattern read as bf16 IS 2^(z*log2e) with
           linearly-interpolated mantissa (max rel err ~3.3%, fine for
           the 2e-2 gate; softmax normalization cancels most of it)
       o_h[dv+1, nq] = [vT_h|1]^T P^T accumulated over nk tiles in PSUM;
       the two heads' PV matmuls run CONCURRENTLY via column tiling
       (tile_position (0,0) / (0,64), disjoint col-groups)
  o_norm = o / denom ; y = Wo@o_norm + (Wo@bv + bo)

Softmax max-subtraction is skipped: logits ~ N(0,1) so exp() cannot
overflow, and softmax is shift-invariant. bv is folded into the output
bias (Wo@bv + bo), computed on-device.

Normalize epilogue per pass: 2 DVE reciprocals (PSUM-direct) -> rec2
[2,512]; one K=2 selector matmul broadcasts both heads' reciprocal rows
across their 32-partition strips; 2 ScalarE copies evacuate raw o; one
DVE tensor_tensor does the [64,512] multiply.

QK/projections run as float32r (full-rate fp32); PV runs BF16.

PSUM budget (8 banks): qk 2x[128,1024]=4 (double-buffered so QK overlaps
exp on both engines), pv 3x[128,512]=3 (accumulate + prev-pass epilogue +
rotation), shared [128,512] slot (projections / rec broadcast / output
projection) = 1.

All engines execute their streams strictly in-order, so the emission is
software-pipelined by hand: each pass's last two PV pairs and its
normalize epilogue are emitted inside the NEXT pass, q/k/v projections
are interleaved into the seams, and each nq-half's output projection +
store overlap the other half's attention.
"""

import numpy as np

import concourse.bass as bass
import concourse.bacc as bacc
import concourse.mybir as mybir
import concourse.tile as tile
from concourse.bass_utils import run_bass_kernel_spmd

F32 = mybir.dt.float32
F32R = mybir.dt.float32r
BF16 = mybir.dt.bfloat16
I16 = mybir.dt.int16
AF = mybir.ActivationFunctionType

P = 128
C = 256          # channels (= Ck = Cv = Co)
CT = 2           # channel tiles of 128
N = 1024         # sequence length (H*W)
NH = 8           # heads
DK = 32          # head dim
SCALE = DK ** -0.5
NQH = 2          # nq halves (512 each; fp32 matmul free-dim limit)
NKT = 8          # nk tiles of 128

LOG2E = 1.4426950408889634
A16 = float(2.0 ** 7 * LOG2E * SCALE)   # schraudolph slope, softmax scale folded
B16 = float(127.0 * 2 ** 7 - 5.5)       # schraudolph offset, C=5.5 tuned

# which nk tiles of each pass run the exp on VectorE (bf16 Schraudolph);
# the rest run on ScalarE (table exp). 26/64 on DVE balances the engines.
DVE_NKS = (
    (1, 3, 5), (1, 3, 5, 7), (1, 3, 5), (1, 3, 5, 7),
    (1, 3, 5), (1, 3, 5, 7), (1, 3, 5), (1, 3, 5, 7),
)


def build_nc(reps=1, pipelined=True):
    nc = bacc.Bacc(None, target_bir_lowering=False, debug=False)

    x_d = nc.dram_tensor("x", [C, N], F32R, kind="ExternalInput")
    wqt_d = nc.dram_tensor("wqt", [C, C], F32R, kind="ExternalInput")
    wkt_d = nc.dram_tensor("wkt", [C, C], F32R, kind="ExternalInput")
    wvt_d = nc.dram_tensor("wvt", [C, C], F32R, kind="ExternalInput")
    wot_d = nc.dram_tensor("wot", [C, C], F32R, kind="ExternalInput")
    sel2_d = nc.dram_tensor("sel2", [DK + 1, 64], F32R, kind="ExternalInput")
    bq_d = nc.dram_tensor("bq", [C], F32, kind="ExternalInput")
    bk_d = nc.dram_tensor("bk", [C], F32, kind="ExternalInput")
    bv_d = nc.dram_tensor("bv", [C], F32R, kind="ExternalInput")
    bo_d = nc.dram_tensor("bo", [C], F32, kind="ExternalInput")
    y_d = nc.dram_tensor("y", [C, N], F32, kind="ExternalOutput")

    with tile.TileContext(nc) as tc:
        with (
            tc.tile_pool(name="const", bufs=1) as cpool,
            tc.tile_pool(name="work", bufs=1) as wpool,
            tc.tile_pool(name="qkpsum", bufs=2, space="PSUM") as qkpool,
            tc.tile_pool(name="pvpsum", bufs=3, space="PSUM") as pvpool,
            tc.tile_pool(name="mmpsum", bufs=1, space="PSUM") as mmpool,
            tc.tile_pool(name="ptpool", bufs=6) as ptpool,
            tc.tile_pool(name="eppool", bufs=3) as eppool,
            tc.tile_pool(name="recpool", bufs=3) as recpool,
        ):
            # ---- loads ----
            w_s = {}
            for name, d in (("q", wqt_d), ("k", wkt_d), ("v", wvt_d), ("o", wot_d)):
                w_s[name] = cpool.tile([P, CT, C], F32R, tag=f"w{name}",
                                       name=f"w{name}")
            x_s = cpool.tile([P, CT, N], F32R)
            xr = x_d[:].rearrange("(t p) n -> p t n", p=P)
            for nh in range(NQH):
                nc.sync.dma_start(
                    x_s[:, 0, nh * 512 : (nh + 1) * 512],
                    xr[:, 0, nh * 512 : (nh + 1) * 512],
                )
                nc.scalar.dma_start(
                    x_s[:, 1, nh * 512 : (nh + 1) * 512],
                    xr[:, 1, nh * 512 : (nh + 1) * 512],
                )
            nc.gpsimd.dma_start(
                w_s["q"][:], wqt_d[:].rearrange("(t p) c -> p t c", p=P)
            )
            nc.gpsimd.dma_start(
                w_s["k"][:], wkt_d[:].rearrange("(t p) c -> p t c", p=P)
            )
            b_s = {}
            for name, d in (("q", bq_d), ("k", bk_d), ("o", bo_d)):
                b_s[name] = cpool.tile([P, CT], F32, tag=f"b{name}",
                                       name=f"b{name}")
                nc.gpsimd.dma_start(b_s[name][:], d[:].rearrange("(t p) -> p t", p=P))
            nc.gpsimd.dma_start(
                w_s["v"][:], wvt_d[:].rearrange("(t p) c -> p t c", p=P)
            )
            nc.gpsimd.dma_start(
                w_s["o"][:], wot_d[:].rearrange("(t p) c -> p t c", p=P)
            )
            # bv duplicated along a free dim of 2: fp32r matmuls need N>=2
            bv2_s = cpool.tile([P, CT, 2], F32R, tag="bv2")
            for j in range(2):
                nc.gpsimd.dma_start(
                    bv2_s[:, :, j], bv_d[:].rearrange("(t p) -> p t", p=P)
                )

            # ---- persistent working tiles ----
            q_s = wpool.tile([P, CT, N], F32R, tag="q")     # [co_p, co_t, n]
            k_s = wpool.tile([P, CT, N], F32R, tag="k")
            # vT in BF16 with a ones column per head: [n_p, n_t, head, dv+1]
            vt_s = wpool.tile([P, NKT, NH, DK + 1], BF16, tag="vt")
            o_s = wpool.tile([P, CT, N], F32R, tag="o")     # normalized attn out
            y_s = wpool.tile([P, CT, N], F32, tag="y")
            bo2_s = wpool.tile([P, CT], F32, tag="bo2")     # Wo@bv + bo

            nc.vector.memset(vt_s[:, :, :, DK : DK + 1], 1.0)
            # selector for the reciprocal broadcast: bcp[0:64] = sel2^T @ rec2.
            # rec2 rows 0 / 32 hold the two heads' reciprocals (engine writes
            # must be 32-partition-aligned); selector rows 1..31 are zero so
            # the zeroed filler rows of rec2 never propagate.
            sel2 = wpool.tile([DK + 1, 64], F32R, tag="sel2")
            nc.gpsimd.dma_start(sel2[:], sel2_d[:])
            rec2 = wpool.tile([DK + 1, 512], F32R, tag="rec2")
            nc.vector.memset(rec2[:].bitcast(F32), 0.0)
            ones1 = wpool.tile([1, 2], F32, tag="ones1")
            nc.vector.memset(ones1[:], 1.0)
            # pull the exp ACT-table load into the load phase
            warm = wpool.tile([1, 2], F32, tag="warm")
            nc.scalar.activation(warm[:], ones1[:], AF.Exp)

            # ---- emission helpers ----
            def qk_proj(name, dst, ct, nh):
                ps = mmpool.tile([P, 512], F32, tag="mm512", name="ps")
                for ci in range(CT):
                    nc.tensor.matmul(
                        ps[:],
                        w_s[name][:, ci, ct * P : (ct + 1) * P],
                        x_s[:, ci, nh * 512 : (nh + 1) * 512],
                        start=(ci == 0),
                        stop=(ci == CT - 1),
                    )
                nc.scalar.activation(
                    dst[:, ct, nh * 512 : (nh + 1) * 512],
                    ps[:],
                    AF.Identity,
                    bias=b_s[name][:, ct : ct + 1],
                )

            def v_proj(nt):
                ps = mmpool.tile([P, 512], F32, tag="mm512", name="ps")
                for ci in range(CT):
                    nc.tensor.matmul(
                        ps[:, 0:C],
                        x_s[:, ci, nt * P : (nt + 1) * P],
                        w_s["v"][:, ci, :],
                        start=(ci == 0),
                        stop=(ci == CT - 1),
                    )
                nc.vector.tensor_copy(
                    vt_s[:, nt, :, 0:DK],
                    ps[:, 0:C].rearrange("p (h d) -> p h d", d=DK),
                )

            def bo2_proj():
                # bo2 = WoT.T @ bv + bo
                for ct in range(CT):
                    ps = mmpool.tile([P, 512], F32, tag="mm512", name="ps")
                    for ci in range(CT):
                        nc.tensor.matmul(
                            ps[:, 0:2],
                            w_s["o"][:, ci, ct * P : (ct + 1) * P],
                            bv2_s[:, ci, :],
                            start=(ci == 0),
                            stop=(ci == CT - 1),
                        )
                    nc.vector.tensor_scalar_add(
                        bo2_s[:, ct : ct + 1], ps[:, 0:1],
                        b_s["o"][:, ct : ct + 1]
                    )

            def pv_pair(pvs, nk, hg, hl0, pt, start, stop):
                # the two heads' PV matmuls occupy disjoint col-groups
                # ({0,1} vs {2,3}) so they run concurrently on the PE
                nc.tensor.matmul(
                    pvs[0 : DK + 1, :],
                    vt_s[:, nk, 4 * hg + hl0, :],
                    pt[:, 0:512],
                    start=start,
                    stop=stop,
                )
                nc.tensor.matmul(
                    pvs[64 : 64 + DK + 1, :],
                    vt_s[:, nk, 4 * hg + hl0 + 1, :],
                    pt[:, 512:1024],
                    start=start,
                    stop=stop,
                    tile_position=(0, 64),
                )

            def attn_pass(nqh, pp, pass_idx, pre_pv=None, tail_prev=None,
                          epi_prev=None, last=False):
                """One pass = 2 heads (4*hg + hl0, +1) x one nq-half.

                The engine streams are strictly in-order, so anything gated
                on this pass's LAST exps is emitted inside the NEXT pass:
                the last two nk's PV pairs as `pv_tail` (at nk==0), the
                normalize epilogue as `epilogue` (at nk==3).
                """
                hg = pp // 2
                hl0 = (pp % 2) * 2
                dve_nks = DVE_NKS[pass_idx]
                pvs = pvpool.tile([P, 512], F32, tag="pv", name="pvs")
                pt_hold = {}
                for nk in range(NKT):
                    qk = qkpool.tile([P, 1024], F32, tag="qk")
                    for j in range(2):
                        hl = hl0 + j
                        nc.tensor.matmul(
                            qk[:, j * 512 : (j + 1) * 512],
                            k_s[hl * DK : (hl + 1) * DK, hg,
                                nk * P : (nk + 1) * P],
                            q_s[hl * DK : (hl + 1) * DK, hg,
                                nqh * 512 : (nqh + 1) * 512],
                            start=True,
                            stop=True,
                            tile_position=(hl * DK, 0),
                        )
                    pt = ptpool.tile([P, 1024], BF16, tag="pt")
                    if nk in dve_nks:
                        with nc.allow_low_precision(reason="bf16 schraudolph"):
                            nc.vector.tensor_scalar(
                                pt[:].bitcast(I16), qk[:], A16, B16,
                                mybir.AluOpType.mult, mybir.AluOpType.add,
                            )
                    else:
                        nc.scalar.activation(pt[:], qk[:], AF.Exp, scale=SCALE)
                    if nk == 0 and tail_prev is not None:
                        tail_prev()
                    if nk == 3 and epi_prev is not None:
                        epi_prev()
                    if pre_pv is not None:
                        pre_pv(nk)
                    if nk < NKT - 2:
                        pv_pair(pvs, nk, hg, hl0, pt, start=(nk == 0),
                                stop=False)
                    else:
                        pt_hold[nk] = pt

                def pv_tail():
                    for nk in (NKT - 2, NKT - 1):
                        pv_pair(pvs, nk, hg, hl0, pt_hold[nk], start=False,
                                stop=(nk == NKT - 1))

                def epilogue():
                    # denominators live in PSUM rows 32 / 96; reciprocal them
                    # into adjacent partitions, broadcast via a K=2 selector
                    # matmul, evacuate raw o on ScalarE, multiply on DVE.
                    with nc.allow_low_precision(reason="f32r recip"):
                        nc.vector.reciprocal(rec2[0:1, :], pvs[DK : DK + 1, :])
                        nc.vector.reciprocal(
                            rec2[DK : DK + 1, :], pvs[64 + DK : 64 + DK + 1, :]
                        )
                    oraw = eppool.tile([64, 512], F32, tag="oraw", name="oraw")
                    nc.scalar.copy(oraw[0:DK, :], pvs[0:DK, :])
                    nc.scalar.copy(oraw[DK : 2 * DK, :], pvs[64 : 64 + DK, :])
                    bcp = mmpool.tile([P, 512], F32, tag="mm512", name="bcp")
                    nc.tensor.matmul(
                        bcp[0:64, :], sel2[:], rec2[:], start=True, stop=True
                    )
                    nc.vector.tensor_tensor(
                        o_s[hl0 * DK : hl0 * DK + 2 * DK, hg,
                            nqh * 512 : (nqh + 1) * 512],
                        oraw[:],
                        bcp[0:64, :],
                        mybir.AluOpType.mult,
                    )

                return pv_tail, epilogue

            def out_proj(nqh, last=False):
                for ct in range(CT):
                    ps = mmpool.tile([P, 512], F32, tag="mm512", name="ps")
                    for i, cv in enumerate((1, 0)):
                        nc.tensor.matmul(
                            ps[:],
                            w_s["o"][:, cv, ct * P : (ct + 1) * P],
                            o_s[:, cv, nqh * 512 : (nqh + 1) * 512],
                            start=(i == 0),
                            stop=(i == CT - 1),
                        )
                    if last:
                        nc.scalar.activation(
                            y_s[:, ct, nqh * 512 : (nqh + 1) * 512],
                            ps[:],
                            AF.Identity,
                            bias=bo2_s[:, ct : ct + 1],
                        )
                    else:
                        nc.vector.tensor_scalar_add(
                            y_s[:, ct, nqh * 512 : (nqh + 1) * 512],
                            ps[:],
                            bo2_s[:, ct : ct + 1],
                        )
                    nc.sync.dma_start(
                        y_d[:].rearrange("(t p) n -> p t n", p=P)[
                            :, ct, nqh * 512 : (nqh + 1) * 512
                        ],
                        y_s[:, ct, nqh * 512 : (nqh + 1) * 512],
                    )

            # ---- emission order: overlap projections with attention ----
            for _rep in range(reps):
                qk_proj("q", q_s, 1, 0)
                qk_proj("k", k_s, 1, 0)
                qk_proj("k", k_s, 1, 1)
                tailp, epip = attn_pass(0, 2, 0, pre_pv=v_proj)
                qk_proj("q", q_s, 0, 0)
                qk_proj("k", k_s, 0, 0)
                tailp, epip = attn_pass(0, 3, 1, tail_prev=tailp, epi_prev=epip)
                qk_proj("k", k_s, 0, 1)
                qk_proj("q", q_s, 1, 1)
                tailp, epip = attn_pass(0, 0, 2, tail_prev=tailp, epi_prev=epip)
                qk_proj("q", q_s, 0, 1)
                bo2_proj()
                tailp, epip = attn_pass(0, 1, 3, tail_prev=tailp, epi_prev=epip)

                def epi_and_oproj0(epip=epip):
                    epip()
                    out_proj(0)

                tailp, epip = attn_pass(1, 2, 4, tail_prev=tailp,
                                        epi_prev=epi_and_oproj0)
                tailp, epip = attn_pass(1, 3, 5, tail_prev=tailp, epi_prev=epip)
                tailp, epip = attn_pass(1, 0, 6, tail_prev=tailp, epi_prev=epip)
                tailp, epip = attn_pass(1, 1, 7, tail_prev=tailp,
                                        epi_prev=epip, last=True)
                tailp()
                epip()
                out_proj(1, last=True)
    nc.compile()
    return nc


_NC = None


def _get_nc():
    global _NC
    if _NC is None:
        _NC = build_nc()
    return _NC


def make_in_maps(x, Wq, bq, Wk, bk, Wv, bv, Wo, bo):
    B = x.shape[0]
    xs = np.ascontiguousarray(x.reshape(B, C, N).astype(np.float32, copy=False))
    sel2 = np.zeros((DK + 1, 64), dtype=np.float32)
    sel2[0, 0:DK] = 1.0
    sel2[DK, DK : 2 * DK] = 1.0
    shared = {
        "sel2": sel2,
        "wqt": np.ascontiguousarray(Wq.T.astype(np.float32, copy=False)),
        "wkt": np.ascontiguousarray(Wk.T.astype(np.float32, copy=False)),
        "wvt": np.ascontiguousarray(Wv.T.astype(np.float32, copy=False)),
        "wot": np.ascontiguousarray(Wo.T.astype(np.float32, copy=False)),
        "bq": np.ascontiguousarray(bq.astype(np.float32, copy=False)),
        "bk": np.ascontiguousarray(bk.astype(np.float32, copy=False)),
        "bv": np.ascontiguousarray(bv.astype(np.float32, copy=False)),
        "bo": np.ascontiguousarray(bo.astype(np.float32, copy=False)),
    }
    return [dict(shared, x=xs[c]) for c in range(B)]


def kernel(x, Wq, bq, Wk, bk, Wv, bv, Wo, bo, **run_kwargs):
    x = np.asarray(x)
    B, _, H, W = x.shape
    in_maps = make_in_maps(
        x, np.asarray(Wq), np.asarray(bq), np.asarray(Wk), np.asarray(bk),
        np.asarray(Wv), np.asarray(bv), np.asarray(Wo), np.asarray(bo),
    )
    res = run_bass_kernel_spmd(_get_nc(), in_maps, core_ids=list(range(B)),
                               **run_kwargs)
    y = np.stack([res.results[c]["y"] for c in range(B)])
    out = y.reshape(B, C, H, W)
    if run_kwargs:
        kernel.last_result = res
    return out


# revision 18
# speedup vs baseline: 1.1716x; 1.1692x over previous
"""MultiHeadAttention (1x1-conv projections) Trainium2 Bass kernel.

Problem: x[8,256,32,32]; q/k/v = conv1x1(x, W*, b*); 8 heads, dk=dv=32;
attention over N=H*W=1024 positions; out = conv1x1(o, Wo, bo).

Sharding: data-parallel over batch -- core c computes batch c.

Per-core dataflow (everything stays on-chip after the initial loads):
  X [256,1024] (C on partitions, 2 tiles of 128)
  q = Wq@X+bq, k = Wk@X+bk       -> [co_part, n]   (co = 32*head+d)
  vT = (Wv@X)^T via x-stationary -> [n_part, co] in BF16 with a ones
       column per head, so the PV matmul also produces the softmax
       denominator for free
  per head pair: S^T[nk,nq] = k_h^T q_h (K=dk=32; the two heads run
       concurrently in distinct PE row strips via tile_position);
       P^T = exp(scale*S^T) -> BF16, split across TWO engines:
         - ScalarE: table exp (ACT), ~1 elem/cycle/lane @1.2GHz
         - VectorE: bf16 Schraudolph in ONE tensor_scalar op:
             i16 = int(z*(2^7*log2e*scale) + (127*2^7 - 5.5));
           the int16 bit pattern read as bf16 IS 2^(z*log2e) with
           linearly-interpolated mantissa (max rel err ~3.3%, fine for
           the 2e-2 gate; softmax normalization cancels most of it)
       o_h[dv+1, nq] = [vT_h|1]^T P^T accumulated over nk tiles in PSUM;
       the two heads' PV matmuls run CONCURRENTLY via column tiling
       (tile_position (0,0) / (0,64), disjoint col-groups)
  o_norm = o / denom ; y = Wo@o_norm + (Wo@bv + bo)

Softmax max-subtraction is skipped: logits ~ N(0,1) so exp() cannot
overflow, and softmax is shift-invariant. bv is folded into the output
bias (Wo@bv + bo), computed on-device.

Normalize epilogue per pass: 2 DVE reciprocals (PSUM-direct) -> rec2
[2,512]; one K=2 selector matmul broadcasts both heads' reciprocal rows
across their 32-partition strips; 2 ScalarE copies evacuate raw o; one
DVE tensor_tensor does the [64,512] multiply.

QK/projections run as float32r (full-rate fp32); PV runs BF16.

PSUM budget (8 banks): qk 2x[128,1024]=4 (double-buffered so QK overlaps
exp on both engines), pv 3x[128,512]=3 (accumulate + prev-pass epilogue +
rotation), shared [128,512] slot (projections / rec broadcast / output
projection) = 1.

All engines execute their streams strictly in-order, so the emission is
software-pipelined by hand: each pass's last two PV pairs and its
normalize epilogue are emitted inside the NEXT pass, q/k/v projections
are interleaved into the seams, and each nq-half's output projection +
store overlap the other half's attention.
"""

import numpy as np

import concourse.bass as bass
import concourse.bacc as bacc
import concourse.mybir as mybir
import concourse.tile as tile
from concourse.bass_utils import run_bass_kernel_spmd

F32 = mybir.dt.float32
F32R = mybir.dt.float32r
BF16 = mybir.dt.bfloat16
I16 = mybir.dt.int16
AF = mybir.ActivationFunctionType

P = 128
C = 256          # channels (= Ck = Cv = Co)
CT = 2           # channel tiles of 128
N = 1024         # sequence length (H*W)
NH = 8           # heads
DK = 32          # head dim
SCALE = DK ** -0.5
NQH = 2          # nq halves (512 each; fp32 matmul free-dim limit)
NKT = 8          # nk tiles of 128

LOG2E = 1.4426950408889634
A16 = float(2.0 ** 7 * LOG2E * SCALE)   # schraudolph slope, softmax scale folded
B16 = float(127.0 * 2 ** 7 - 5.5)       # schraudolph offset, C=5.5 tuned

# which nk tiles of each pass run the exp on VectorE (bf16 Schraudolph);
# the rest on ScalarE (table exp). HW-measured: DVE tile 657ns vs ACT
# 1034ns, so DVE takes 34/64; pass 0 takes fewer (it also runs v_proj
# evacuations on DVE).
DVE_NKS = (
    (2, 5), (0, 2, 4, 6, 7), (0, 2, 4, 6), (0, 2, 4, 6, 7),
    (0, 2, 4, 6), (0, 2, 4, 6, 7), (0, 2, 4, 6), (0, 2, 4, 6, 7),
)


def build_nc(reps=1, pipelined=True):
    nc = bacc.Bacc(None, target_bir_lowering=False, debug=False)

    x_d = nc.dram_tensor("x", [C, N], F32R, kind="ExternalInput")
    wqt_d = nc.dram_tensor("wqt", [C, C], F32R, kind="ExternalInput")
    wkt_d = nc.dram_tensor("wkt", [C, C], F32R, kind="ExternalInput")
    wvt_d = nc.dram_tensor("wvt", [C, C], F32R, kind="ExternalInput")
    wot_d = nc.dram_tensor("wot", [C, C], F32R, kind="ExternalInput")
    sel2_d = nc.dram_tensor("sel2", [DK + 1, 64], F32R, kind="ExternalInput")
    bq_d = nc.dram_tensor("bq", [C], F32, kind="ExternalInput")
    bk_d = nc.dram_tensor("bk", [C], F32, kind="ExternalInput")
    bv_d = nc.dram_tensor("bv", [C], F32R, kind="ExternalInput")
    bo_d = nc.dram_tensor("bo", [C], F32, kind="ExternalInput")
    y_d = nc.dram_tensor("y", [C, N], F32, kind="ExternalOutput")

    with tile.TileContext(nc) as tc:
        with (
            tc.tile_pool(name="const", bufs=1) as cpool,
            tc.tile_pool(name="work", bufs=1) as wpool,
            tc.tile_pool(name="qkpsum", bufs=2, space="PSUM") as qkpool,
            tc.tile_pool(name="pvpsum", bufs=2, space="PSUM") as pvpool,
            tc.tile_pool(name="mmpsum", bufs=2, space="PSUM") as mmpool,
            tc.tile_pool(name="ptpool", bufs=6) as ptpool,
            tc.tile_pool(name="eppool", bufs=3) as eppool,
            tc.tile_pool(name="recpool", bufs=3) as recpool,
        ):
            # ---- loads ----
            w_s = {}
            for name, d in (("q", wqt_d), ("k", wkt_d), ("v", wvt_d), ("o", wot_d)):
                w_s[name] = cpool.tile([P, CT, C], F32R, tag=f"w{name}",
                                       name=f"w{name}")
            x_s = cpool.tile([P, CT, N], F32R)
            xr = x_d[:].rearrange("(t p) n -> p t n", p=P)
            for nh in range(NQH):
                nc.sync.dma_start(
                    x_s[:, 0, nh * 512 : (nh + 1) * 512],
                    xr[:, 0, nh * 512 : (nh + 1) * 512],
                )
                nc.scalar.dma_start(
                    x_s[:, 1, nh * 512 : (nh + 1) * 512],
                    xr[:, 1, nh * 512 : (nh + 1) * 512],
                )
            nc.gpsimd.dma_start(
                w_s["q"][:], wqt_d[:].rearrange("(t p) c -> p t c", p=P)
            )
            nc.gpsimd.dma_start(
                w_s["k"][:], wkt_d[:].rearrange("(t p) c -> p t c", p=P)
            )
            b_s = {}
            for name, d in (("q", bq_d), ("k", bk_d), ("o", bo_d)):
                b_s[name] = cpool.tile([P, CT], F32, tag=f"b{name}",
                                       name=f"b{name}")
                nc.gpsimd.dma_start(b_s[name][:], d[:].rearrange("(t p) -> p t", p=P))
            nc.gpsimd.dma_start(
                w_s["v"][:], wvt_d[:].rearrange("(t p) c -> p t c", p=P)
            )
            nc.gpsimd.dma_start(
                w_s["o"][:], wot_d[:].rearrange("(t p) c -> p t c", p=P)
            )
            # bv duplicated along a free dim of 2: fp32r matmuls need N>=2
            bv2_s = cpool.tile([P, CT, 2], F32R, tag="bv2")
            for j in range(2):
                nc.gpsimd.dma_start(
                    bv2_s[:, :, j], bv_d[:].rearrange("(t p) -> p t", p=P)
                )

            # ---- persistent working tiles ----
            q_s = wpool.tile([P, CT, N], F32R, tag="q")     # [co_p, co_t, n]
            k_s = wpool.tile([P, CT, N], F32R, tag="k")
            # vT in BF16 with a ones column per head: [n_p, n_t, head, dv+1]
            vt_s = wpool.tile([P, NKT, NH, DK + 1], BF16, tag="vt")
            o_s = wpool.tile([P, CT, N], F32R, tag="o")     # normalized attn out
            y_s = wpool.tile([P, CT, N], F32, tag="y")
            bo2_s = wpool.tile([P, CT], F32, tag="bo2")     # Wo@bv + bo

            nc.vector.memset(vt_s[:, :, :, DK : DK + 1], 1.0)
            # selector for the reciprocal broadcast: bcp[0:64] = sel2^T @ rec2.
            # rec2 rows 0 / 32 hold the two heads' reciprocals (engine writes
            # must be 32-partition-aligned); selector rows 1..31 are zero so
            # the zeroed filler rows of rec2 never propagate.
            sel2 = wpool.tile([DK + 1, 64], F32R, tag="sel2")
            nc.gpsimd.dma_start(sel2[:], sel2_d[:])
            rec2 = wpool.tile([DK + 1, 512], F32R, tag="rec2")
            nc.vector.memset(rec2[:].bitcast(F32), 0.0)
            ones1 = wpool.tile([1, 2], F32, tag="ones1")
            nc.vector.memset(ones1[:], 1.0)
            # pull the exp ACT-table load into the load phase
            warm = wpool.tile([1, 2], F32, tag="warm")
            nc.scalar.activation(warm[:], ones1[:], AF.Exp)

            # ---- emission helpers ----
            def qk_proj(name, dst, ct, nh):
                ps = mmpool.tile([P, 512], F32, tag="mm512", name="ps")
                for ci in range(CT):
                    nc.tensor.matmul(
                        ps[:],
                        w_s[name][:, ci, ct * P : (ct + 1) * P],
                        x_s[:, ci, nh * 512 : (nh + 1) * 512],
                        start=(ci == 0),
                        stop=(ci == CT - 1),
                    )
                nc.scalar.activation(
                    dst[:, ct, nh * 512 : (nh + 1) * 512],
                    ps[:],
                    AF.Identity,
                    bias=b_s[name][:, ct : ct + 1],
                )

            def v_proj(nt):
                ps = mmpool.tile([P, 512], F32, tag="mm512", name="ps")
                for ci in range(CT):
                    nc.tensor.matmul(
                        ps[:, 0:C],
                        x_s[:, ci, nt * P : (nt + 1) * P],
                        w_s["v"][:, ci, :],
                        start=(ci == 0),
                        stop=(ci == CT - 1),
                    )
                nc.vector.tensor_copy(
                    vt_s[:, nt, :, 0:DK],
                    ps[:, 0:C].rearrange("p (h d) -> p h d", d=DK),
                )

            def bo2_proj():
                # bo2 = WoT.T @ bv + bo
                for ct in range(CT):
                    ps = mmpool.tile([P, 512], F32, tag="mm512", name="ps")
                    for ci in range(CT):
                        nc.tensor.matmul(
                            ps[:, 0:2],
                            w_s["o"][:, ci, ct * P : (ct + 1) * P],
                            bv2_s[:, ci, :],
                            start=(ci == 0),
                            stop=(ci == CT - 1),
                        )
                    nc.vector.tensor_scalar_add(
                        bo2_s[:, ct : ct + 1], ps[:, 0:1],
                        b_s["o"][:, ct : ct + 1]
                    )

            def pv_pair(pvs, nk, hg, hl0, pt, start, stop):
                # the two heads' PV matmuls occupy disjoint col-groups
                # ({0,1} vs {2,3}) so they run concurrently on the PE
                nc.tensor.matmul(
                    pvs[0 : DK + 1, :],
                    vt_s[:, nk, 4 * hg + hl0, :],
                    pt[:, 0:512],
                    start=start,
                    stop=stop,
                )
                nc.tensor.matmul(
                    pvs[64 : 64 + DK + 1, :],
                    vt_s[:, nk, 4 * hg + hl0 + 1, :],
                    pt[:, 512:1024],
                    start=start,
                    stop=stop,
                    tile_position=(0, 64),
                )

            def attn_pass(nqh, pp, pass_idx, pre_pv=None, tail_prev=None,
                          epi_prev=None, last=False):
                """One pass = 2 heads (4*hg + hl0, +1) x one nq-half.

                The engine streams are strictly in-order, so anything gated
                on this pass's LAST exps is emitted inside the NEXT pass:
                the last two nk's PV pairs as `pv_tail` (at nk==0), the
                normalize epilogue as `epilogue` (at nk==3).
                """
                hg = pp // 2
                hl0 = (pp % 2) * 2
                dve_nks = DVE_NKS[pass_idx]
                pvs = pvpool.tile([P, 512], F32, tag="pv", name="pvs")
                pt_hold = {}
                for nk in range(NKT):
                    qk = qkpool.tile([P, 1024], F32, tag="qk")
                    for j in range(2):
                        hl = hl0 + j
                        nc.tensor.matmul(
                            qk[:, j * 512 : (j + 1) * 512],
                            k_s[hl * DK : (hl + 1) * DK, hg,
                                nk * P : (nk + 1) * P],
                            q_s[hl * DK : (hl + 1) * DK, hg,
                                nqh * 512 : (nqh + 1) * 512],
                            start=True,
                            stop=True,
                            tile_position=(hl * DK, 0),
                        )
                    pt = ptpool.tile([P, 1024], BF16, tag="pt")
                    if nk in dve_nks:
                        with nc.allow_low_precision(reason="bf16 schraudolph"):
                            nc.vector.tensor_scalar(
                                pt[:].bitcast(I16), qk[:], A16, B16,
                                mybir.AluOpType.mult, mybir.AluOpType.add,
                            )
                    else:
                        nc.scalar.activation(pt[:], qk[:], AF.Exp, scale=SCALE)
                    if nk == 0 and tail_prev is not None:
                        tail_prev()
                    if nk == 3 and epi_prev is not None:
                        epi_prev()
                    if pre_pv is not None:
                        pre_pv(nk)
                    if nk < NKT - 2:
                        pv_pair(pvs, nk, hg, hl0, pt, start=(nk == 0),
                                stop=False)
                    else:
                        pt_hold[nk] = pt

                def pv_tail():
                    for nk in (NKT - 2, NKT - 1):
                        pv_pair(pvs, nk, hg, hl0, pt_hold[nk], start=False,
                                stop=(nk == NKT - 1))

                def epilogue():
                    # denominators live in PSUM rows 32 / 96; reciprocal them
                    # into adjacent partitions, broadcast via a K=2 selector
                    # matmul, evacuate raw o on ScalarE, multiply on DVE.
                    with nc.allow_low_precision(reason="f32r recip"):
                        nc.vector.reciprocal(rec2[0:1, :], pvs[DK : DK + 1, :])
                        nc.vector.reciprocal(
                            rec2[DK : DK + 1, :], pvs[64 + DK : 64 + DK + 1, :]
                        )
                    oraw = eppool.tile([64, 512], F32, tag="oraw", name="oraw")
                    nc.scalar.copy(oraw[0:DK, :], pvs[0:DK, :])
                    nc.scalar.copy(oraw[DK : 2 * DK, :], pvs[64 : 64 + DK, :])
                    bcp = mmpool.tile([P, 512], F32, tag="mm512", name="bcp")
                    nc.tensor.matmul(
                        bcp[0:64, :], sel2[:], rec2[:], start=True, stop=True
                    )
                    nc.vector.tensor_tensor(
                        o_s[hl0 * DK : hl0 * DK + 2 * DK, hg,
                            nqh * 512 : (nqh + 1) * 512],
                        oraw[:],
                        bcp[0:64, :],
                        mybir.AluOpType.mult,
                    )

                return pv_tail, epilogue

            def out_proj(nqh, last=False):
                for ct in range(CT):
                    ps = mmpool.tile([P, 512], F32, tag="mm512", name="ps")
                    for i, cv in enumerate((1, 0)):
                        nc.tensor.matmul(
                            ps[:],
                            w_s["o"][:, cv, ct * P : (ct + 1) * P],
                            o_s[:, cv, nqh * 512 : (nqh + 1) * 512],
                            start=(i == 0),
                            stop=(i == CT - 1),
                        )
                    if last:
                        nc.scalar.activation(
                            y_s[:, ct, nqh * 512 : (nqh + 1) * 512],
                            ps[:],
                            AF.Identity,
                            bias=bo2_s[:, ct : ct + 1],
                        )
                    else:
                        nc.vector.tensor_scalar_add(
                            y_s[:, ct, nqh * 512 : (nqh + 1) * 512],
                            ps[:],
                            bo2_s[:, ct : ct + 1],
                        )
                    nc.sync.dma_start(
                        y_d[:].rearrange("(t p) n -> p t n", p=P)[
                            :, ct, nqh * 512 : (nqh + 1) * 512
                        ],
                        y_s[:, ct, nqh * 512 : (nqh + 1) * 512],
                    )

            # ---- emission order: overlap projections with attention ----
            for _rep in range(reps):
                qk_proj("q", q_s, 1, 0)
                qk_proj("k", k_s, 1, 0)
                qk_proj("k", k_s, 1, 1)
                tailp, epip = attn_pass(0, 2, 0, pre_pv=v_proj)
                qk_proj("q", q_s, 0, 0)
                qk_proj("k", k_s, 0, 0)
                tailp, epip = attn_pass(0, 3, 1, tail_prev=tailp, epi_prev=epip)
                qk_proj("k", k_s, 0, 1)
                qk_proj("q", q_s, 1, 1)
                tailp, epip = attn_pass(0, 0, 2, tail_prev=tailp, epi_prev=epip)
                qk_proj("q", q_s, 0, 1)
                bo2_proj()
                tailp, epip = attn_pass(0, 1, 3, tail_prev=tailp, epi_prev=epip)

                def epi_and_oproj0(epip=epip):
                    epip()
                    out_proj(0)

                tailp, epip = attn_pass(1, 2, 4, tail_prev=tailp,
                                        epi_prev=epi_and_oproj0)
                tailp, epip = attn_pass(1, 3, 5, tail_prev=tailp, epi_prev=epip)
                tailp, epip = attn_pass(1, 0, 6, tail_prev=tailp, epi_prev=epip)
                tailp, epip = attn_pass(1, 1, 7, tail_prev=tailp,
                                        epi_prev=epip, last=True)
                tailp()
                epip()
                out_proj(1, last=True)
    nc.compile()
    return nc


_NC = None


def _get_nc():
    global _NC
    if _NC is None:
        _NC = build_nc()
    return _NC


def make_in_maps(x, Wq, bq, Wk, bk, Wv, bv, Wo, bo):
    B = x.shape[0]
    xs = np.ascontiguousarray(x.reshape(B, C, N).astype(np.float32, copy=False))
    sel2 = np.zeros((DK + 1, 64), dtype=np.float32)
    sel2[0, 0:DK] = 1.0
    sel2[DK, DK : 2 * DK] = 1.0
    shared = {
        "sel2": sel2,
        "wqt": np.ascontiguousarray(Wq.T.astype(np.float32, copy=False)),
        "wkt": np.ascontiguousarray(Wk.T.astype(np.float32, copy=False)),
        "wvt": np.ascontiguousarray(Wv.T.astype(np.float32, copy=False)),
        "wot": np.ascontiguousarray(Wo.T.astype(np.float32, copy=False)),
        "bq": np.ascontiguousarray(bq.astype(np.float32, copy=False)),
        "bk": np.ascontiguousarray(bk.astype(np.float32, copy=False)),
        "bv": np.ascontiguousarray(bv.astype(np.float32, copy=False)),
        "bo": np.ascontiguousarray(bo.astype(np.float32, copy=False)),
    }
    return [dict(shared, x=xs[c]) for c in range(B)]


def kernel(x, Wq, bq, Wk, bk, Wv, bv, Wo, bo, **run_kwargs):
    x = np.asarray(x)
    B, _, H, W = x.shape
    in_maps = make_in_maps(
        x, np.asarray(Wq), np.asarray(bq), np.asarray(Wk), np.asarray(bk),
        np.asarray(Wv), np.asarray(bv), np.asarray(Wo), np.asarray(bo),
    )
    res = run_bass_kernel_spmd(_get_nc(), in_maps, core_ids=list(range(B)),
                               **run_kwargs)
    y = np.stack([res.results[c]["y"] for c in range(B)])
    out = y.reshape(B, C, H, W)
    if run_kwargs:
        kernel.last_result = res
    return out
